# revision 10
# baseline (speedup 1.0000x reference)
# Trainium2 Bass kernel for ByteCombineCNN (conv byte-encoder + highway + projection).
#
# v2: restructured from the v1 baseline guided by TimelineSim:
#   - PSUM fully double-buffered in 8 banks: conv_ps+ht_ps share a tag
#     (rotation c0,c1,ht over 2x2-bank slots), pg (p,g,p,g over 2x1), o (2x1)
#   - highway biases ride the matmul via a ones row (row 112) of hT and a
#     bias row of hwT; hT/hT_next/hT_fin are explicit ping-pong tiles whose
#     rows 96:128 are memset to 1.0 once, pre-loop
#   - relu(p) optionally on Pool (parallel with ACT sigmoid)
#   - optional half-group (256-sample) highway/proj pipelining
#   - transposes + stores issue from SP; loads stay on gpsimd (cast DMA)
import numpy as np
import ml_dtypes

bf16 = ml_dtypes.bfloat16

B, T, BYTE_LEN, EMB = 8, 4096, 8, 64
FILTERS = [(1, 4), (2, 8), (3, 12), (4, 16), (5, 20), (6, 24), (7, 28)]
NPOS = [BYTE_LEN - w + 1 for w, _ in FILTERS]
LAST_DIM = 112
OUT_DIM = 512
FEAT = BYTE_LEN * EMB          # 512
CONV_COLS = sum(c * p for (w, c), p in zip(FILTERS, NPOS))  # 448
# positions padded to even count (odd-p filters duplicate position 0, which
# leaves the max unchanged) so the first maxpool level is one pairwise
# tensor_max; f7 (2 positions) finishes in that level, f1..f6 finish in a
# small segmented reduce over 4/4/3/3/2/2 pair-maxes.
P_PAD = [p + (p % 2) for p in NPOS]            # [8, 8, 6, 6, 4, 4, 2]
CONV_COLS_P = sum(c * p for ((w, c), p) in zip(FILTERS, P_PAD))  # 496
# 2-class padding: f1-f4 at p=8, f5-f7 at p=4 -> only two merged reduces/pr
P_PAD2 = [8, 8, 8, 8, 4, 4, 4]
CONV_COLS_P2 = sum(c * p for ((w, c), p) in zip(FILTERS, P_PAD2))  # 608
N_CORES = 8
S_PER_CORE = B * T // N_CORES  # 4096
GROUP = 512                    # samples per group
NG = S_PER_CORE // GROUP       # 8
NST = GROUP // 128             # 4 subtiles per group

_cache = {}


def _build(reps=1):
    import os
    import concourse.mybir as mybir
    import concourse.tile as tile
    from concourse import bacc
    from contextlib import ExitStack

    dt = mybir.dt
    nc = bacc.Bacc("TRN2", target_bir_lowering=False, debug=False)

    # Debug/ablation knobs only honored when KDEV=1.
    dev = os.environ.get("KDEV", "0") == "1"
    def _env(name, default):
        return os.environ.get(name, default) if dev else default
    ktreng = _env("KTRENG", "sync")    # transpose issue engine
    ktt = _env("KTT", "ddd")           # highway (sub, mul, add) engines: d/p
    kcopy = _env("KCOPY", "aada")      # proj copy engine per subtile: a/d (gpsimd cannot read PSUM)
    krelu = _env("KRELU", "act")       # highway relu engine: act/pool/dve
    khalf = int(_env("KHALF", "0"))    # split highway/proj into 256-sample halves
    ksb = int(_env("KSB", "2"))        # sbuf pool depth
    ktr = int(_env("KTR", "1"))        # dma-transposes per group
    kemit = _env("KEMIT", "pipe")      # emission order: pipelined or group-major
    kper = int(_env("KPER", "0"))      # modulo-schedule period ns (0 = off)
    kpoff = int(_env("KPOFF", "0"))    # tail-stage offset ns
    kcoff = int(_env("KCOFF", "0"))    # conv-stage offset ns
    kprl = int(_env("KPRL", "0"))      # priority boost (insts) for load stage
    kprc = int(_env("KPRC", "80"))     # priority boost (insts) for conv stage
    kpair = int(_env("KPAIR", "1"))    # 0: per-filter; 1: 4-class pads; 2: 2-class
    kop2 = int(_env("KOP2", "0"))      # 1: pair proj outputs 2-per-PSUM-tile
    kops = int(_env("KOPS", "0"))      # 1: DMA output straight from PSUM (no osb)
    kprx = int(_env("KPRX", "60"))     # deprioritize proj copies+store by N insts
    kprh = int(_env("KPRH", "0"))      # boost highway acts+TT chain by N insts
    kwarm = int(_env("KWARM", "28"))   # PE-ramp warmup transposes (0 = off)
    kswp = int(_env("KSWP", "0"))      # 1: emit gate mm+sigmoid before proj mm+relu
    kft = int(_env("KFT", "0"))        # fill-tail boost: tails of first 3 groups
    kst2 = int(_env("KST2", "4"))      # output store split: 1/2/4 pieces
    krh2 = int(_env("KRH2", "0"))      # split relu_hT into 2 halves
    kps2 = int(_env("KPS2", "0"))      # PSUM: 1/2: conv 3x2; 2: ht+pg+o one tag

    # features pre-transposed AND pre-cast to bf16 on the host:
    # [feat%128, g, st, kc, samp%128] -- halves the HBM read vs f32 and
    # frees the load from the gpsimd cast-DGE requirement
    feat = nc.dram_tensor("features", [128, NG, NST, 4, 128], dt.bfloat16, kind="ExternalInput").ap()
    ccols = {0: CONV_COLS, 1: CONV_COLS_P, 2: CONV_COLS_P2}[kpair]
    wbig_d = nc.dram_tensor("wbig", [128, 4 * ccols], dt.bfloat16, kind="ExternalInput").ap()
    hwT_d = nc.dram_tensor("hwT", [128, 448], dt.bfloat16, kind="ExternalInput").ap()
    pwT_d = nc.dram_tensor("pwT", [128, 512], dt.bfloat16, kind="ExternalInput").ap()
    cbias_d = nc.dram_tensor("cbias", [112, 1], dt.float32, kind="ExternalInput").ap()
    ident_d = nc.dram_tensor("ident", [128, 128], dt.bfloat16, kind="ExternalInput").ap()
    outp = nc.dram_tensor("out", [S_PER_CORE, OUT_DIM], dt.float32, kind="ExternalOutput").ap()

    outv = outp.rearrange("(g st p) o -> g p st o", st=NST, p=128)


    def eng(name):
        return {"sync": nc.sync, "gpsimd": nc.gpsimd, "scalar": nc.scalar,
                "vector": nc.vector}[name]

    from contextlib import nullcontext as _nullctx

    with tile.TileContext(nc) as tc, ExitStack() as ctx:
        const = ctx.enter_context(tc.tile_pool(name="const", bufs=1))
        # ident + cbias first: the PE/ACT warmups below depend on them
        ident_sb = const.tile([128, 128], dt.bfloat16, name="ident_sb")
        nc.sync.dma_start(out=ident_sb[:], in_=ident_d)
        cbias_sb = const.tile([112, 1], dt.float32, name="cbias_sb")
        nc.sync.dma_start(out=cbias_sb[:], in_=cbias_d)
        wbig_sb = const.tile([128, 4, ccols], dt.bfloat16, name="wbig_sb")
        wbig_v = wbig_d.rearrange("p (k c) -> p k c", k=4)
        if int(_env("KWB4", "0")):
            for k4 in range(4):
                nc.sync.dma_start(out=wbig_sb[:, k4], in_=wbig_v[:, k4])
        else:
            nc.sync.dma_start(out=wbig_sb[:], in_=wbig_v)
        hwT_sb = const.tile([128, 448], dt.bfloat16, name="hwT_sb")
        nc.sync.dma_start(out=hwT_sb[:], in_=hwT_d)
        pwT_sb = const.tile([128, 512], dt.bfloat16, name="pwT_sb")
        nc.sync.dma_start(out=pwT_sb[:], in_=pwT_d)

        xt_pool = ctx.enter_context(tc.tile_pool(name="xt", bufs=int(_env("KBXT", str(ksb + 1)))))
        hraw_pool = ctx.enter_context(tc.tile_pool(name="hraw", bufs=int(_env("KBHR", str(ksb)))))
        act_pool = ctx.enter_context(tc.tile_pool(name="act", bufs=int(_env("KBAC", str(ksb)))))
        out_pool = ctx.enter_context(tc.tile_pool(name="outsb", bufs=int(_env("KBOS", str(ksb)))))
        # PSUM (8 banks): conv+ht share a 2x(2-bank) tag (rotation c0,c1,ht);
        # pg rotates p,g,p,g over 2x1; o rotates the 4 proj outputs over 2x1.
        convht_pool = ctx.enter_context(tc.tile_pool(name="convht", bufs=3 if kps2 else 2, space="PSUM"))
        if kps2 != 2:
            pg_ps_pool = ctx.enter_context(tc.tile_pool(name="pg_ps", bufs=2, space="PSUM"))
        o_ps_pool = ctx.enter_context(tc.tile_pool(name="o_ps", bufs=2, space="PSUM"))

        # hT ping-pong tiles: rows 96:128 hold 1.0 (row 112 is the matmul
        # bias row against hwT/pwT row 112; rows 113+ hit zero weight rows).
        # Rows 0:112 are rewritten every use; 112:128 persist from here.
        hts = []
        for nm in ("h0a", "h0b", "h1a", "h1b", "h2a", "h2b"):
            ft = const.tile([128, GROUP], dt.bfloat16, name=f"ht_{nm}")
            nc.vector.memset(ft[96:128, :], 1.0)
            hts.append(ft)
        ht_ping = {l: (hts[2 * l], hts[2 * l + 1]) for l in range(3)}

        # Warmups: ~3us of back-to-back PE work ramps the p-state to max
        # before group 0's conv arrives; two tiny activations preload both
        # ACT function-table sets so neither LoadActFuncSet lands mid-chain.
        if kwarm:
            warm_ps = (o_ps_pool if kps2 == 2 else pg_ps_pool).tile(
                [128, 128], dt.bfloat16, name="warm_ps",
                tag="o" if kps2 == 2 else "pg", bufs=2)
            for _ in range(kwarm):
                nc.tensor.transpose(warm_ps[:], ident_sb[:], ident_sb[:])
            warm_act = const.tile([1, 2], dt.float32, name="warm_act")
            nc.scalar.activation(warm_act[0:1, 0:1], cbias_sb[0:1, 0:1],
                                 mybir.ActivationFunctionType.Relu)
            nc.scalar.activation(warm_act[0:1, 1:2], cbias_sb[0:1, 0:1],
                                 mybir.ActivationFunctionType.Sigmoid)

        nsub = 2
        gseq = [gg for _ in range(reps) for gg in range(NG)]
        NTOT = len(gseq)

        def stage_load(g):
            # ---- load group, already feature-major bf16 ----
            xt = xt_pool.tile([128, NST, 4, 128], dt.bfloat16, name="xt")
            nld = int(_env("KLD2", "2"))
            wl = NST // nld
            for sp in range(nld):
                eng(_env("KLDE", "gpsimd")).dma_start(
                    out=xt[:, sp * wl:(sp + 1) * wl], in_=feat[:, g, sp * wl:(sp + 1) * wl])
            return xt

        def stage_conv(xt):
            # ---- conv as dense matmul + maxpool (all on DVE; gpsimd cannot
            # touch PSUM on TRN2) ----
            hraw = hraw_pool.tile([128, NST, LAST_DIM], dt.bfloat16, name="hraw")
            for pr in range(NST // nsub):
                conv_ps = convht_pool.tile([128, nsub, max(512, ccols)], dt.float32,
                                           name="conv_ps", tag="cht")
                for sub in range(nsub):
                    st = pr * nsub + sub
                    for kc in range(4):
                        nc.tensor.matmul(
                            conv_ps[:, sub, 0:ccols],
                            lhsT=xt[:, st, kc, :],
                            rhs=wbig_sb[:, kc, :],
                            start=(kc == 0),
                            stop=(kc == 3),
                        )
                sl = slice(pr * nsub, (pr + 1) * nsub)
                if kpair:
                    # even-padded layout: filters with equal padded p merge
                    # into one single-PSUM-input segmented reduce
                    off = 0
                    offc = 0
                    groups = ((12, 8), (28, 6), (44, 4), (28, 2)) if kpair == 1 else ((40, 8), (72, 4))
                    for cc, pp in groups:
                        seg = conv_ps[:, :, off:off + cc * pp].rearrange(
                            "a b (c p) -> a b c p", p=pp
                        )
                        nc.vector.tensor_reduce(
                            out=hraw[:, sl, offc:offc + cc],
                            in_=seg,
                            axis=mybir.AxisListType.X,
                            op=mybir.AluOpType.max,
                        )
                        off += cc * pp
                        offc += cc
                    continue
                off = 0
                offc = 0
                for fi, ((w, c), p_i) in enumerate(zip(FILTERS, NPOS)):
                    seg = conv_ps[:, :, off:off + c * p_i].rearrange(
                        "a b (cc p) -> a b cc p", p=p_i
                    )
                    nc.vector.tensor_reduce(
                        out=hraw[:, sl, offc:offc + c],
                        in_=seg,
                        axis=mybir.AxisListType.X,
                        op=mybir.AluOpType.max,
                    )
                    off += c * p_i
                    offc += c
            return hraw

        def stage_tail(idx, g, hraw):
            hT = stage_hw(idx, hraw)
            stage_proj(idx, g, hT)

        def stage_hw(idx, hraw):
            # KHALF=2: split only the fill/drain groups (latency, not rate)
            NH = 2 if (khalf == 1 or (khalf == 2 and (idx == 0 or idx == NTOT - 1))) else 1
            HW = GROUP // NH
            # ---- transpose h to [c, s]; conv bias+relu on ACT ----
            ht_ps = (o_ps_pool if kps2 else convht_pool).tile(
                [112, NST, 128], dt.bfloat16,
                name="ht_ps", tag="o" if kps2 else "cht", bufs=2 if kps2 else None)

            for st in range(NST):
                nc.tensor.transpose(ht_ps[:, st, :], hraw[:, st, :], ident_sb[:])
            hT = ht_ping[0][idx % 2]
            nrh = max(NH, 1 + krh2)
            for h in range(nrh):
                hw_r = GROUP // nrh
                nc.scalar.activation(
                    hT[0:112, h * hw_r:(h + 1) * hw_r],
                    ht_ps.rearrange("a b c -> a (b c)")[:, h * hw_r:(h + 1) * hw_r],
                    mybir.ActivationFunctionType.Relu, bias=cbias_sb[:],
                )

            # ---- two highway layers (bias via ones row 112 against hwT row 112) ----
            for l in range(2):
                pgp = o_ps_pool if kps2 == 2 else pg_ps_pool
                pgt = "o" if kps2 == 2 else "pg"
                p_ps = pgp.tile([112, GROUP], dt.float32, name="p_ps", tag=pgt, bufs=2)
                g_ps = pgp.tile([112, GROUP], dt.float32, name="g_ps", tag=pgt, bufs=2)
                hT_next = ht_ping[l + 1][idx % 2]
                for h in range(NH):
                    sl = slice(h * HW, (h + 1) * HW)
                    mm_p = lambda: nc.tensor.matmul(
                        p_ps[:, sl], lhsT=hwT_sb[:, l * 224:l * 224 + 112],
                        rhs=hT[:, sl], start=True, stop=True)
                    mm_g = lambda: nc.tensor.matmul(
                        g_ps[:, sl], lhsT=hwT_sb[:, l * 224 + 112:l * 224 + 224],
                        rhs=hT[:, sl], start=True, stop=True)
                    if kswp:
                        mm_g(); mm_p()
                    else:
                        mm_p(); mm_g()
                rp = act_pool.tile([112, GROUP], dt.bfloat16, name="rp")
                gs = act_pool.tile([112, GROUP], dt.bfloat16, name="gs")
                d = act_pool.tile([112, GROUP], dt.bfloat16, name="d")
                e = act_pool.tile([112, GROUP], dt.bfloat16, name="e")
                tteng = [nc.vector if ch == "d" else nc.gpsimd for ch in ktt]
                hw_prio = ctx2 = None
                if kprh:
                    from contextlib import ExitStack as _ES
                    ctx2 = _ES()
                    ctx2.enter_context(tc.high_priority(kprh))
                for h in range(NH):
                    sl = slice(h * HW, (h + 1) * HW)
                    act_r = lambda: nc.scalar.activation(
                        rp[:, sl], p_ps[:, sl], mybir.ActivationFunctionType.Relu)
                    act_s = lambda: nc.scalar.activation(
                        gs[:, sl], g_ps[:, sl], mybir.ActivationFunctionType.Sigmoid)
                    if kswp:
                        act_s(); act_r()
                    else:
                        act_r(); act_s()
                    tteng[0].tensor_sub(d[:, sl], hT[0:112, sl], rp[:, sl])
                    tteng[1].tensor_mul(e[:, sl], gs[:, sl], d[:, sl])
                    tteng[2].tensor_add(hT_next[0:112, sl], e[:, sl], rp[:, sl])
                if ctx2 is not None:
                    ctx2.close()
                hT = hT_next
            return hT

        def stage_proj(idx, g, hT):
            # ---- projection: out[s, 512] directly (hT stationary, bias row 112) ----
            osb = out_pool.tile([128, NST, OUT_DIM], dt.float32, name="osb")
            if kop2:
                # two subtiles per 2-bank PSUM tile; one bigger copy per pair
                for half in range(NST // 2):
                    o_ps = o_ps_pool.tile([128, 2, OUT_DIM], dt.float32,
                                          name="o_ps2", tag="o2", bufs=1)
                    for j in range(2):
                        st = half * 2 + j
                        nc.tensor.matmul(o_ps[:, j], lhsT=hT[:, st * 128:(st + 1) * 128],
                                         rhs=pwT_sb[:], start=True, stop=True)
                    ce = kcopy[half % len(kcopy)]
                    with (tc.high_priority(-kprx) if kprx else _nullctx()):
                        if ce == "a":
                            nc.scalar.copy(out=osb[:, half * 2:(half + 1) * 2, :], in_=o_ps[:])
                        else:
                            nc.vector.tensor_copy(out=osb[:, half * 2:(half + 1) * 2, :], in_=o_ps[:])
            else:
                for st in range(NST):
                    o_ps = o_ps_pool.tile([128, OUT_DIM], dt.float32, name="o_ps", tag="o", bufs=2)
                    nc.tensor.matmul(o_ps[:], lhsT=hT[:, st * 128:(st + 1) * 128],
                                     rhs=pwT_sb[:], start=True, stop=True)
                    ce = kcopy[st % len(kcopy)]
                    with (tc.high_priority(-kprx) if kprx else _nullctx()):
                        if ce == "a":
                            nc.scalar.copy(out=osb[:, st, :], in_=o_ps[:])
                        else:
                            nc.vector.tensor_copy(out=osb[:, st, :], in_=o_ps[:])
            with (tc.high_priority(-kprx) if kprx else _nullctx()):
                nsp = max(1, kst2)
                w = NST // nsp
                for sp in range(nsp):
                    nc.sync.dma_start(out=outv[g, :, sp * w:(sp + 1) * w],
                                      in_=osb[:, sp * w:(sp + 1) * w])

        if kemit in ("pipe", "pipe4"):
            # software-pipelined emission: program order (= scheduler priority)
            # matches the steady-state overlap load(k+dl) | conv(k+1) | tail(k);
            # pipe4 additionally trails proj+store one slot behind the highway
            kdl = int(_env("KDL", "1"))    # load lead (slots) ahead of conv
            korder = _env("KORD", "lct")   # intra-slot emission order
            if kemit == "pipe4":
                korder = _env("KORD", "lchp")
            xts, hraws, hts_r = {}, {}, {}
            for s in range(NTOT + 2 + kdl + (1 if kemit == "pipe4" else 0)):
                for ch in korder:
                    if ch == "l" and s < NTOT:
                        xts[s] = stage_load(gseq[s])
                    elif ch == "c" and 0 <= s - kdl < NTOT:
                        hraws[s - kdl] = stage_conv(xts.pop(s - kdl))
                    elif ch == "t" and 0 <= s - kdl - 1 < NTOT:
                        ti = s - kdl - 1
                        boost = kft * max(0, 3 - ti)
                        with (tc.high_priority(boost) if boost else _nullctx()):
                            stage_tail(ti, gseq[ti], hraws.pop(ti))
                    elif ch == "h" and 0 <= s - kdl - 1 < NTOT:
                        hi = s - kdl - 1
                        hts_r[hi] = stage_hw(hi, hraws.pop(hi))
                    elif ch == "p" and 0 <= s - kdl - 2 < NTOT:
                        pi = s - kdl - 2
                        stage_proj(pi, gseq[pi], hts_r.pop(pi))
        else:
            from contextlib import nullcontext
            for s, g in enumerate(gseq):
                wait = (lambda off: tc.tile_wait_until((s * kper + off) * 1e-6)) if kper else (lambda off: nullcontext())
                with wait(0), (tc.high_priority(kprl) if kprl else nullcontext()):
                    xt = stage_load(g)
                with wait(kcoff), (tc.high_priority(kprc) if kprc else nullcontext()):
                    hraw = stage_conv(xt)
                with wait(kpoff):
                    stage_tail(s, g, hraw)

    nc.compile()
    return nc


def _prep_weights(inputs, pair=1):
    ncols = {0: CONV_COLS, 1: CONV_COLS_P, 2: CONV_COLS_P2}[pair]
    pads = {0: NPOS, 1: P_PAD, 2: P_PAD2}[pair]
    W = np.zeros((FEAT, ncols), np.float32)
    cb = np.zeros(LAST_DIM, np.float32)
    off = 0
    offc = 0
    for i, ((w, c), p_i) in enumerate(zip(FILTERS, NPOS)):
        pp = pads[i]
        cw = np.asarray(inputs[f"conv_w{i+1}"], np.float32)  # [c, EMB, w]
        for p in range(p_i):
            for k in range(w):
                byte = p + k
                W[byte * EMB:(byte + 1) * EMB, off + p:off + c * pp:pp] = cw[:, :, k].T
        for d in range(pp - p_i):  # pad slots duplicate real positions (max unchanged)
            srcp = d % p_i
            for k in range(w):
                byte = srcp + k
                W[byte * EMB:(byte + 1) * EMB, off + p_i + d:off + c * pp:pp] = cw[:, :, k].T
        cb[offc:offc + c] = np.asarray(inputs[f"conv_b{i+1}"], np.float32)
        off += c * pp
        offc += c
    wbig = np.ascontiguousarray(
        W.reshape(4, 128, ncols).transpose(1, 0, 2).reshape(128, 4 * ncols)
    ).astype(bf16)
    # hwT: [128, 448]; rows 0:112 the weights, row 112 the biases (paired with
    # the constant-1 row 112 of hT), rows 113:127 zero (hT rows 113:127 are 1).
    hwT = np.zeros((128, 448), np.float32)
    hwT[:112, :224] = np.asarray(inputs["hw_w1"], np.float32).T.reshape(112, 224)
    hwT[:112, 224:] = np.asarray(inputs["hw_w2"], np.float32).T.reshape(112, 224)
    hwT[112, :224] = np.asarray(inputs["hw_b1"], np.float32)
    hwT[112, 224:] = np.asarray(inputs["hw_b2"], np.float32)
    hwT = np.ascontiguousarray(hwT).astype(bf16)
    pwT = np.zeros((128, 512), np.float32)
    pwT[:112] = np.asarray(inputs["proj_w"], np.float32).T
    pwT[112] = np.asarray(inputs["proj_b"], np.float32)
    pwT = np.ascontiguousarray(pwT).astype(bf16)
    return wbig, hwT, pwT, cb.reshape(112, 1)


def _prep_features(inputs):
    feats = np.asarray(inputs["features"], np.float32).reshape(B * T, FEAT)
    cores = []
    for c in range(N_CORES):
        fc = feats[c * S_PER_CORE:(c + 1) * S_PER_CORE]
        # [s_total, f] -> [p, g, st, kc, s]  (f = kc*128 + p; s_total = ((g*NST)+st)*128 + s)
        ft = fc.reshape(NG, NST, 128, 4, 128).transpose(4, 0, 1, 3, 2)
        cores.append(np.ascontiguousarray(ft).astype(bf16))
    return cores


def kernel(**inputs) -> np.ndarray:
    from concourse.bass_utils import run_bass_kernel_spmd

    if "nc" not in _cache:
        _cache["nc"] = _build()
    nc = _cache["nc"]

    import os
    dev = os.environ.get("KDEV", "0") == "1"
    pair = int(os.environ.get("KPAIR", "1") if dev else "1")
    wbig, hwT, pwT, cb = _prep_weights(inputs, pair=pair)
    ident = np.eye(128, dtype=bf16)
    featsT = _prep_features(inputs)

    in_maps = []
    for c in range(N_CORES):
        in_maps.append({
            "features": featsT[c],
            "wbig": wbig, "hwT": hwT, "pwT": pwT, "cbias": cb,
            "ident": ident,
        })
    res = run_bass_kernel_spmd(nc, in_maps, core_ids=list(range(N_CORES)))
    out = np.concatenate([res.results[c]["out"] for c in range(N_CORES)], 0)
    return out.reshape(B, T, OUT_DIM)
